# revision 31
# baseline (speedup 1.0000x reference)
"""Trainium2 Bass kernel for EHRCPCC loss (Pearson corr between condensed
pairwise L2 distances and a label-disagreement indicator over all B*(B-1)/2
upper-triangle pairs).

Strategy (8 NeuronCores, symmetric wrapped-circulant decomposition):
  * Pearson needs only the moments Sx, Sxx, Sxy, Sy, N over the P pairs.
      - Sy = n0*n1, Syy = Sy (binary labels) -- exact on host.
      - Sxx = sum_{i<j} d2_ij = B*sum(sq) - ||sum_i x_i||^2 -- exact f64 host.
      - Sx, Sxy need the actual sqrt(d2) values -> computed on device.
  * The distance matrix is symmetric, so only ~half of it is evaluated:
    the 4096 rows form 16 blocks of 256; row-block r evaluates the column
    blocks (r+o) mod 16 for o = 0..K(r), K(r) = 8 for r < 8 else 7. Every
    unordered block pair is covered exactly once, and core c owning blocks
    {c, c+8} gets a strip of 2304 + 2048 = 4352 columns -- identical shape
    on every core, so one SPMD program serves all 8.
  * Full row sums A[m] (and label-weighted C[m]) are reconstructed on the
    host as (right part from the owner's strip row sums) + (left part from
    the column sums of the covering strips). Column sums (plain and
    label-weighted) come from a [ones; labels] stationary matmul over the
    bf16 x tiles on the otherwise idle PE.
  * x = sqrt(sq_i + sq_j - 2*g_ij + EPS): the +EPS keeps the (excluded)
    diagonal, where fp rounding can make d2 slightly negative, out of NaN
    territory without a clamp op. A constant shift cancels in Pearson
    centering; the diagonal's known sqrt(EPS) contribution is subtracted on
    the host.
  * Matmul inputs are bf16; all host-side moments are computed from the SAME
    bf16-rounded values so device and host stay consistent. The stationary
    operand is pre-scaled by -2 (exact in bf16) so PSUM holds -2*gram.
  * Per [128, <=1024] chunk: 2-4 matmuls, 1 DVE tensor_add (sq_j broadcast),
    1 ACT Sqrt(+ per-partition sq_i bias, accum_out row sum, bf16 out),
    1 DVE tensor_tensor_reduce with a broadcast 0/1 label mask (accum_out
    label-weighted row sum). No label sorting or boundary partials needed.
"""

import ml_dtypes
import numpy as np

import concourse.bass as bass
import concourse.mybir as mybir
from concourse import bacc
from concourse.bass_utils import run_bass_kernel_spmd
from concourse.tile import TileContext

B = 4096
D = 256
NCORES = 8
NRB = 16                   # row blocks
RB = B // NRB              # 256 rows per block
WA = 9 * RB                # strip A width (blocks c..c+8)
WB = 8 * RB                # strip B width (blocks c+8..c+15)
W = WA + WB                # 4352 strip columns per core
EPS = 1e-2

# (local_start, width) chunks; A strip then B strip
CHUNKS = [
    (0, [(0, 1024), (1024, 1024), (2048, 256)]),
    (1, [(WA, 1024), (WA + 1024, 1024)]),
]
NCH = [3, 2]               # chunks per strip


def _npieces(w):
    return [(o, min(512, w - o)) for o in range(0, w, 512)]


def _racc_col(q, ch):
    return q * 3 + ch  # A-strips use 3 cols, B-strips use 2 of 3


_PROGRAM = None
LAST_RESULTS = None  # BassKernelResults of the most recent run (for profiling)


def _build_program():
    f32 = mybir.dt.float32
    bf16 = mybir.dt.bfloat16
    nc = bacc.Bacc(None, target_bir_lowering=False, num_swdge_queues=2)

    xt_d = nc.dram_tensor("xt", [2 * 128, W], bf16, kind="ExternalInput")
    lt_d = nc.dram_tensor("lt", [128, 1024], bf16, kind="ExternalInput")
    sqx_d = nc.dram_tensor("sqx", [3, W], bf16, kind="ExternalInput")
    lb_d = nc.dram_tensor("lb", [1, W], bf16, kind="ExternalInput")
    sqr_d = nc.dram_tensor("sqr", [128, 4], f32, kind="ExternalInput")
    ol_d = nc.dram_tensor("ol", [128, 8], bf16, kind="ExternalInput")
    out_d = nc.dram_tensor("out", [128, 24], f32, kind="ExternalOutput")
    cso_d = nc.dram_tensor("cso", [2, W], f32, kind="ExternalOutput")

    with TileContext(nc) as tc:
        with (
            tc.tile_pool(name="big", bufs=1) as big,
            tc.tile_pool(name="xp", bufs=4) as xp,
            tc.tile_pool(name="xlp", bufs=2) as xlp,
            tc.tile_pool(name="psum", bufs=3, space="PSUM") as pp,
            tc.tile_pool(name="csp", bufs=2, space="PSUM") as cp,
        ):
            xts = [
                big.tile([128, W], bf16, tag=f"xt{k}", name=f"xt{k}")
                for k in range(2)
            ]
            lts = big.tile([128, 1024], bf16, tag="lts")
            sqx_s = big.tile([3, W], bf16, tag="sqx")
            ones3 = big.tile([3, 128], bf16, tag="ones3")
            lbb = big.tile([128, W], bf16, tag="lbb")
            sqr_s = big.tile([128, 4], f32, tag="sqr")
            ol_s = big.tile([128, 8], bf16, tag="ol")
            racc = big.tile([128, 12], f32, tag="racc")
            cacc = big.tile([128, 12], f32, tag="cacc")
            csacc = big.tile([2, W], f32, tag="csacc")

            nc.vector.memset(racc, 0.0)
            nc.vector.memset(cacc, 0.0)
            nc.vector.memset(ones3, 1.0)
            # Input loads spread across the three DMA-capable queues.
            nc.gpsimd.dma_start(out=lts, in_=lt_d[:, :])
            nc.gpsimd.dma_start(out=sqr_s, in_=sqr_d[:, :])
            nc.gpsimd.dma_start(out=ol_s, in_=ol_d[:, :])
            nc.gpsimd.dma_start(out=sqx_s, in_=sqx_d[:, :])
            lb_ap = lb_d[:, :]
            for (st, w) in ((0, WA), (WA, WB)):
                bl = bass.AP(
                    tensor=lb_ap.tensor, offset=lb_ap.offset + st,
                    ap=[[0, 128], [1, w]],
                )
                nc.gpsimd.dma_start(out=lbb[:, st:st + w], in_=bl)
            for (st, w) in ((0, WA), (WA, WB)):
                nc.sync.dma_start(out=xts[0][:, st:st + w], in_=xt_d[0:128, st:st + w])
                nc.scalar.dma_start(
                    out=xts[1][:, st:st + w], in_=xt_d[128:256, st:st + w]
                )

            for b, chunks in CHUNKS:
                for ch, (cst, w) in enumerate(chunks):
                    xtiles = []
                    for pt in range(2):
                        q = 2 * b + pt
                        ps = pp.tile([128, 1024], f32, tag="ps")
                        for (no, nw) in _npieces(w):
                            for k in range(2):
                                nc.tensor.matmul(
                                    ps[:, no:no + nw],
                                    lts[:, 512 * k + 128 * q:512 * k + 128 * (q + 1)],
                                    xts[k][:, cst + no:cst + no + nw],
                                    start=(k == 0), stop=False,
                                )
                            # sq_j enters via 3 extra contraction rows
                            # (exact bf16 3-term split), so PSUM = sq_j - 2g.
                            nc.tensor.matmul(
                                ps[:, no:no + nw], ones3,
                                sqx_s[:, cst + no:cst + no + nw],
                                start=False, stop=True,
                            )
                        x = xp.tile([128, 1024], bf16, tag="x")
                        rc = _racc_col(q, ch)
                        nc.scalar.activation(
                            x[:, :w], ps[:, :w], mybir.ActivationFunctionType.Sqrt,
                            bias=sqr_s[:, q:q + 1], scale=1.0,
                            accum_out=racc[:, rc:rc + 1],
                        )
                        xl = xlp.tile([128, 1024], bf16, tag="xl")
                        nc.gpsimd.tensor_mul(
                            xl[:, :w], x[:, :w], lbb[:, cst:cst + w]
                        )
                        nc.vector.reduce_sum(
                            out=cacc[:, rc:rc + 1], in_=xl[:, :w],
                            axis=mybir.AxisListType.X,
                        )
                        xtiles.append(x)
                    # column sums: [ones; labels]^T @ x over both 128-row tiles
                    for (no, nw) in _npieces(w):
                        cs = cp.tile([2, 512], f32, tag="cs")
                        for pt in range(2):
                            q = 2 * b + pt
                            nc.tensor.matmul(
                                cs[:, :nw], ol_s[:, 2 * q:2 * q + 2],
                                xtiles[pt][:, no:no + nw],
                                start=(pt == 0), stop=(pt == 1),
                            )
                        nc.vector.tensor_copy(
                            csacc[:, cst + no:cst + no + nw], cs[:, :nw]
                        )

            nc.sync.dma_start(out=cso_d[:, :], in_=csacc)
            nc.sync.dma_start(out=out_d[:, 0:12], in_=racc)
            nc.sync.dma_start(out=out_d[:, 12:24], in_=cacc)

    nc.finalize()
    return nc


def _strip_gcols(r):
    k = 8 if r < 8 else 7
    return np.concatenate(
        [np.arange(((r + o) % NRB) * RB, ((r + o) % NRB) * RB + RB)
         for o in range(k + 1)]
    )


def kernel(representations: np.ndarray, labels: np.ndarray) -> np.ndarray:
    X = np.ascontiguousarray(representations, dtype=np.float32)
    lab = np.asarray(labels).astype(np.int64)
    assert X.shape == (B, D)

    n0 = int((lab == 0).sum())
    n1 = B - n0
    Pn = B * (B - 1) // 2

    Sy = float(n0) * float(n1)
    vy = Sy - Sy * Sy / Pn
    if vy <= 0.0:
        # Zero label variance -> corr is NaN -> reference returns 1.0.
        return np.asarray(1.0, dtype=np.float32)

    # Device matmuls consume bf16: compute all host-side moments from the
    # SAME rounded values so the (excluded) diagonal d2 stays ~0 and Sxx is
    # consistent with the device's sqrt(d2).
    Xb = X.astype(ml_dtypes.bfloat16)
    X64 = Xb.astype(np.float64)
    sq64 = np.einsum("ij,ij->i", X64, X64)
    s64 = X64.sum(axis=0)
    Sxx = B * sq64.sum() - s64 @ s64  # == sum_{i<j} ||x_i - x_j||^2, exact

    global _PROGRAM
    if _PROGRAM is None:
        _PROGRAM = _build_program()
    nc = _PROGRAM

    XT = np.ascontiguousarray(Xb.T)               # [256, 4096] bf16
    XT2 = (XT * ml_dtypes.bfloat16(-2.0))         # pre-scaled stationary
    sq32 = sq64.astype(np.float32)
    labf = lab.astype(np.float32)

    in_maps = []
    gcols_by_r = {r: _strip_gcols(r) for r in range(NRB)}
    # Exact 3-term bf16 decomposition of sq (columns side): s1+s2+s3 == sq32
    sd1 = sq32.astype(ml_dtypes.bfloat16)
    rr1 = sq32 - sd1.astype(np.float32)
    sd2 = rr1.astype(ml_dtypes.bfloat16)
    sd3 = (rr1 - sd2.astype(np.float32)).astype(ml_dtypes.bfloat16)
    sqx3 = np.stack([sd1, sd2, sd3], axis=0)          # [3, B] bf16

    for c in range(NCORES):
        gc = np.concatenate([gcols_by_r[c], gcols_by_r[c + 8]])
        xt = np.ascontiguousarray(XT[:, gc])                      # [256, W]
        sqx_in = np.ascontiguousarray(sqx3[:, gc])                # [3, W]
        lb_in = np.ascontiguousarray(
            labf[gc].astype(ml_dtypes.bfloat16).reshape(1, W)
        )
        # lt packed [128, 1024]: K-chunk k, (block,pt) combo q at
        # cols [512k + 128q, 512k + 128(q+1))
        lt = np.empty((128, 1024), dtype=ml_dtypes.bfloat16)
        sqr = np.empty((128, 4), dtype=np.float32)
        ol = np.zeros((128, 8), dtype=ml_dtypes.bfloat16)
        for b, r in ((0, c), (1, c + 8)):
            for pt in range(2):
                q = 2 * b + pt
                rows = slice(r * RB + pt * 128, r * RB + (pt + 1) * 128)
                for k in range(2):
                    lt[:, 512 * k + 128 * q:512 * k + 128 * (q + 1)] = \
                        XT2[k * 128:(k + 1) * 128, rows]
                sqr[:, q] = sq32[rows] + np.float32(EPS)
                ol[:, 2 * q] = ml_dtypes.bfloat16(1.0)
                ol[:, 2 * q + 1] = labf[rows].astype(ml_dtypes.bfloat16)
        in_maps.append({
            "xt": xt, "lt": np.ascontiguousarray(lt), "sqx": sqx_in,
            "lb": lb_in, "sqr": np.ascontiguousarray(sqr),
            "ol": np.ascontiguousarray(ol),
        })

    res = run_bass_kernel_spmd(nc, in_maps, core_ids=list(range(NCORES)))
    global LAST_RESULTS
    LAST_RESULTS = res

    # ---- host combine (f64) ----
    rowpart = np.zeros(B)
    cpart = np.zeros(B)
    CS = np.zeros((NRB, B))
    LCS = np.zeros((NRB, B))
    for c in range(NCORES):
        out = res.results[c]["out"].astype(np.float64)   # [128, 24]
        cso = res.results[c]["cso"].astype(np.float64)   # [2, W]
        racc, cacc = out[:, :12], out[:, 12:24]
        for b, r in ((0, c), (1, c + 8)):
            gc = gcols_by_r[r]
            lo = 0 if b == 0 else WA
            CS[r, gc] = cso[0, lo:lo + len(gc)]
            LCS[r, gc] = cso[1, lo:lo + len(gc)]
            for pt in range(2):
                q = 2 * b + pt
                rows = slice(r * RB + pt * 128, r * RB + (pt + 1) * 128)
                cols = [_racc_col(q, ch) for ch in range(NCH[b])]
                rowpart[rows] = racc[:, cols].sum(axis=1)
                cpart[rows] = cacc[:, cols].sum(axis=1)

    A = rowpart.copy()
    C = cpart.copy()
    for g in range(NRB):
        rows = slice(g * RB, (g + 1) * RB)
        for rp in range(NRB):
            if rp == g:
                continue
            if (g - rp) % NRB <= (8 if rp < 8 else 7):
                A[rows] += CS[rp, g * RB:(g + 1) * RB]
                C[rows] += LCS[rp, g * RB:(g + 1) * RB]

    lab64 = lab.astype(np.float64)
    SxF = A.sum() - B * np.sqrt(EPS)  # remove diagonal sqrt(EPS) terms
    SxyF = (C + lab64 * (A - 2.0 * C)).sum()
    Sx = SxF / 2.0
    Sxy = SxyF / 2.0

    cov = Sxy - Sx * Sy / Pn
    vx = Sxx - Sx * Sx / Pn
    corr = cov / np.sqrt(vx * vy)
    loss = 1.0 - corr
    if not np.isfinite(loss):
        loss = 1.0
    return np.asarray(loss, dtype=np.float32)
